# revision 8
# baseline (speedup 1.0000x reference)
"""Trainium2 Bass kernel for nn_CrossAttention_65051574665735.

Cross-attention block (MQA, shared K/V head) + parallel SwiGLU FF.
Data-parallel over B*N rows across 8 NeuronCores: core c handles batch c//4,
rows (c%4)*512. Context + weights replicated (weights pre-cast to bf16 with the
layernorm scale g and the 1/sqrt(dh) attention scale folded in on the host).
No cross-core collectives; the host concatenates the 8 output slices.
"""

import sys

if "/opt/trn_rl_repo" not in sys.path:
    sys.path.insert(0, "/opt/trn_rl_repo")

import numpy as np
import ml_dtypes

import concourse.bass as bass
import concourse.tile as tile
from concourse import mybir, bacc
from concourse.masks import make_identity

F32 = mybir.dt.float32
BF16 = mybir.dt.bfloat16

B, N, J = 2, 2048, 2048
DIM, HEADS, DH = 1024, 16, 64
INNER = HEADS * DH
FF = 4 * DIM
EPS = 1e-5
N_CORES = 8
R = B * N // N_CORES  # 512 rows per core
KT = DIM // 128  # 8 contraction tiles over dim
RT = R // 128  # 4 row tiles
CT = J // 128  # 16 context row tiles
FT = FF // 128  # 32 ff tiles


def _layernorm_transpose(
    nc, tc, pools, src_dram, rows, xnT_tiles, bias_tile, tag
):
    """LN rows of src_dram ([rows, DIM] f32) and write transposed bf16 tiles.

    xnT_tiles: list of KT sbuf tiles [128, rows] bf16 (dim-major).
    bias_tile: optional [128, DIM] f32 sbuf tile holding broadcast norm_b.
    """
    ln_pool, stats_pool, psum_tr, ident, eps_tile = pools
    n_tiles = rows // 128
    for t in range(n_tiles):
        x_t = ln_pool.tile([128, DIM], F32, tag="ln_x")
        nc.gpsimd.dma_start(x_t[:], src_dram[t * 128 : (t + 1) * 128, :])
        stats = stats_pool.tile(
            [128, 2, nc.vector.BN_STATS_DIM], F32, tag="st"
        )
        nc.vector.bn_stats(stats[:, 0, :], x_t[:, 0:512])
        nc.vector.bn_stats(stats[:, 1, :], x_t[:, 512:1024])
        mv = stats_pool.tile([128, nc.vector.BN_AGGR_DIM], F32, tag="mv")
        nc.vector.bn_aggr(mv[:], stats[:])
        rstd = stats_pool.tile([128, 1], F32, tag="rs")
        nc.scalar.activation(
            rstd[:],
            mv[:, 1:2],
            mybir.ActivationFunctionType.Sqrt,
            bias=eps_tile[:],
        )
        nc.vector.reciprocal(rstd[:], rstd[:])
        xn_t = ln_pool.tile([128, DIM], BF16, tag="ln_xn")
        nc.vector.tensor_scalar(
            out=xn_t[:],
            in0=x_t[:],
            scalar1=mv[:, 0:1],
            scalar2=rstd[:],
            op0=mybir.AluOpType.subtract,
            op1=mybir.AluOpType.mult,
        )
        if bias_tile is not None:
            nc.vector.tensor_add(xn_t[:], xn_t[:], bias_tile[:])
        for k in range(KT):
            ps = psum_tr.tile([128, 128], BF16, tag="tr")
            nc.tensor.transpose(
                ps[:], xn_t[:, k * 128 : (k + 1) * 128], ident[:]
            )
            nc.vector.tensor_copy(
                xnT_tiles[k][:, t * 128 : (t + 1) * 128], ps[:]
            )


def build_kernel(x_bias_nonzero: bool, c_bias_nonzero: bool):
    nc = bacc.Bacc(
        "TRN2", target_bir_lowering=False, debug=False, num_devices=N_CORES
    )
    d_x = nc.dram_tensor("x", [R, DIM], F32, kind="ExternalInput").ap()
    d_ctx = nc.dram_tensor("ctx", [J, DIM], F32, kind="ExternalInput").ap()
    d_wq = nc.dram_tensor("wq", [DIM, INNER], BF16, kind="ExternalInput").ap()
    d_wkv = nc.dram_tensor("wkv", [DIM, 2 * DH], BF16, kind="ExternalInput").ap()
    d_wo = nc.dram_tensor("wo", [INNER, DIM], BF16, kind="ExternalInput").ap()
    d_wff1 = nc.dram_tensor("wff1", [DIM, 2 * FF], BF16, kind="ExternalInput").ap()
    d_wff2 = nc.dram_tensor("wff2", [FF, DIM], BF16, kind="ExternalInput").ap()
    d_xb = (
        nc.dram_tensor("xb", [1, DIM], F32, kind="ExternalInput").ap()
        if x_bias_nonzero
        else None
    )
    d_cb = (
        nc.dram_tensor("cb", [1, DIM], F32, kind="ExternalInput").ap()
        if c_bias_nonzero
        else None
    )
    d_out = nc.dram_tensor("out", [R, DIM], F32, kind="ExternalOutput").ap()

    with tile.TileContext(nc) as tc:
        with (
            tc.tile_pool(name="consts", bufs=1) as consts,
            tc.tile_pool(name="persist", bufs=1) as persist,
            tc.tile_pool(name="ln", bufs=3) as ln_pool,
            tc.tile_pool(name="stats", bufs=3) as stats_pool,
        ):
            ident = consts.tile([128, 128], BF16)
            make_identity(nc, ident)
            eps_tile = consts.tile([128, 1], F32, tag="eps")
            nc.vector.memset(eps_tile[:], EPS)

            xb_tile = cb_tile = None
            if d_xb is not None:
                xb_tile = consts.tile([128, DIM], F32, tag="xb")
                nc.gpsimd.dma_start(
                    xb_tile[:],
                    bass.AP(
                        tensor=d_xb.tensor, offset=d_xb.offset,
                        ap=[[0, 128]] + d_xb.ap[1:],
                    ),
                )
            if d_cb is not None:
                cb_tile = consts.tile([128, DIM], F32, tag="cb")
                nc.gpsimd.dma_start(
                    cb_tile[:],
                    bass.AP(
                        tensor=d_cb.tensor, offset=d_cb.offset,
                        ap=[[0, 128]] + d_cb.ap[1:],
                    ),
                )

            xnT = [
                persist.tile([128, R], BF16, tag=f"xnT{k}", name=f"xnT{k}") for k in range(KT)
            ]
            kT = persist.tile([128, J], BF16, tag="kT")
            vo = [
                persist.tile([128, DH + 1], BF16, tag=f"vo{j}", name=f"vo{j}")
                for j in range(CT)
            ]
            aoT = [
                persist.tile([128, R], BF16, tag=f"aoT{k}", name=f"aoT{k}") for k in range(KT)
            ]

            # ---- Phase A: layernorms, transposes, K/V projection ----
            with (
                tc.tile_pool(name="cnT", bufs=1) as cnT_pool,
                tc.tile_pool(name="psA", bufs=2, space="PSUM") as psum_tr,
                tc.tile_pool(name="psKV", bufs=2, space="PSUM") as psum_kv,
                tc.tile_pool(name="wkv", bufs=1) as wkv_pool,
                tc.tile_pool(name="vstage", bufs=2) as vstage,
            ):
                pools = (ln_pool, stats_pool, psum_tr, ident, eps_tile)
                _layernorm_transpose(
                    nc, tc, pools, d_x, R, xnT, xb_tile, "x"
                )
                cnT = [
                    cnT_pool.tile([128, J], BF16, tag=f"cnT{k}", name=f"cnT{k}")
                    for k in range(KT)
                ]
                _layernorm_transpose(
                    nc, tc, pools, d_ctx, J, cnT, cb_tile, "c"
                )

                wkv_sb = [
                    wkv_pool.tile([128, 2 * DH], BF16, tag=f"wkv{k}", name=f"wkv{k}")
                    for k in range(KT)
                ]
                for k in range(KT):
                    nc.gpsimd.dma_start(
                        wkv_sb[k][:], d_wkv[k * 128 : (k + 1) * 128, :]
                    )
                for jc in range(CT):
                    nc.vector.memset(vo[jc][:, DH : DH + 1], 1.0)

                for c in range(J // 512):
                    kv_ps = psum_kv.tile([128, 512], F32, tag="kv")
                    for k in range(KT):
                        nc.tensor.matmul(
                            kv_ps[:],
                            wkv_sb[k][:],
                            cnT[k][:, c * 512 : (c + 1) * 512],
                            start=(k == 0),
                            stop=(k == KT - 1),
                        )
                    # rows 0:64 = kT chunk; rows 64:128 = vT chunk
                    nc.scalar.copy(
                        kT[0:DH, c * 512 : (c + 1) * 512], kv_ps[0:DH, :]
                    )
                    nc.gpsimd.dma_start(
                        kT[DH:128, c * 512 : (c + 1) * 512],
                        kT[0:DH, c * 512 : (c + 1) * 512],
                    )
                    vT_sb = vstage.tile([128, 512], BF16, tag="vT")
                    nc.vector.tensor_copy(vT_sb[DH:128, :], kv_ps[DH:128, :])
                    for j4 in range(4):
                        jc = c * 4 + j4
                        vps = psum_tr.tile([128, DH], BF16, tag="vtr")
                        nc.tensor.transpose(
                            vps[:],
                            vT_sb[DH:128, j4 * 128 : (j4 + 1) * 128],
                            ident[DH:128, DH:128],
                        )
                        nc.vector.tensor_copy(vo[jc][:, 0:DH], vps[:])

            # ---- Phase B: attention ----
            with (
                tc.tile_pool(name="wq", bufs=1) as wq_pool,
                tc.tile_pool(name="attnT", bufs=20) as attn_pool,
                tc.tile_pool(name="qT", bufs=2) as qT_pool,
                tc.tile_pool(name="smx", bufs=2) as smx_pool,
                tc.tile_pool(name="psQ", bufs=2, space="PSUM") as psum_q,
                tc.tile_pool(name="psS", bufs=3, space="PSUM") as psum_s,
                tc.tile_pool(name="psAV", bufs=2, space="PSUM") as psum_av,
            ):
                wq_sb = [
                    wq_pool.tile([128, INNER], BF16, tag=f"wq{k}", name=f"wq{k}")
                    for k in range(KT)
                ]
                for k in range(KT):
                    nc.gpsimd.dma_start(
                        wq_sb[k][:], d_wq[k * 128 : (k + 1) * 128, :]
                    )
                for hp in range(HEADS // 2):
                    q_ps = psum_q.tile([128, R], F32, tag="q")
                    for k in range(KT):
                        nc.tensor.matmul(
                            q_ps[:],
                            wq_sb[k][:, hp * 128 : (hp + 1) * 128],
                            xnT[k][:],
                            start=(k == 0),
                            stop=(k == KT - 1),
                        )
                    qT_sb = qT_pool.tile([128, R], BF16, tag="qT")
                    nc.scalar.copy(qT_sb[:], q_ps[:])
                    for h2 in range(2):
                        qh = qT_sb[h2 * 64 : (h2 + 1) * 64, :]
                        attnT = []
                        for jc in range(CT):
                            s_ps = psum_s.tile([128, R], F32, tag="sim")
                            nc.tensor.matmul(
                                s_ps[:],
                                kT[
                                    h2 * DH : (h2 + 1) * DH,
                                    jc * 128 : (jc + 1) * 128,
                                ],
                                qh,
                                start=True,
                                stop=True,
                            )
                            a_sb = attn_pool.tile([128, R], BF16, tag="attnT")
                            nc.scalar.activation(
                                a_sb[:],
                                s_ps[:],
                                mybir.ActivationFunctionType.Exp,
                            )
                            attnT.append(a_sb)
                        av_ps = psum_av.tile([DH + 1, R], F32, tag="av")
                        for jc in range(CT):
                            nc.tensor.matmul(
                                av_ps[:],
                                vo[jc][:],
                                attnT[jc][:],
                                start=(jc == 0),
                                stop=(jc == CT - 1),
                            )
                        recip_t = smx_pool.tile([DH + 1, R], F32, tag="recipt")
                        nc.vector.reciprocal(
                            recip_t[DH : DH + 1, :], av_ps[DH : DH + 1, :]
                        )
                        recip = smx_pool.tile([1, R], F32, tag="recip")
                        nc.gpsimd.dma_start(recip[:], recip_t[DH : DH + 1, :])
                        rbc = smx_pool.tile([DH, R], F32, tag="rbc")
                        nc.gpsimd.partition_broadcast(rbc[:], recip[:])
                        if h2 == 0:
                            nc.vector.tensor_mul(
                                aoT[hp][0:DH, :], av_ps[0:DH, :], rbc[:]
                            )
                        else:
                            tmp = smx_pool.tile([DH, R], BF16, tag="aotmp")
                            nc.vector.tensor_mul(
                                tmp[:], av_ps[0:DH, :], rbc[:]
                            )
                            nc.gpsimd.dma_start(
                                aoT[hp][DH:128, :], tmp[:]
                            )

            # ---- Phase C: FF up-proj + SwiGLU ----
            hT = [
                persist.tile([128, R], BF16, tag=f"hT{f}", name=f"hT{f}") for f in range(FT)
            ]
            with (
                tc.tile_pool(name="wff1", bufs=24) as wff1_pool,
                tc.tile_pool(name="sg", bufs=3) as sg_pool,
                tc.tile_pool(name="psF", bufs=2, space="PSUM") as psum_f,
            ):
                for g in range(4):
                    w1a = []
                    w1g = []
                    for k in range(KT):
                        ta = wff1_pool.tile([128, 1024], BF16, tag="w1")
                        nc.gpsimd.dma_start(
                            ta[:],
                            d_wff1[
                                k * 128 : (k + 1) * 128,
                                g * 1024 : (g + 1) * 1024,
                            ],
                        )
                        w1a.append(ta)
                        tg = wff1_pool.tile([128, 1024], BF16, tag="w1")
                        nc.gpsimd.dma_start(
                            tg[:],
                            d_wff1[
                                k * 128 : (k + 1) * 128,
                                FF + g * 1024 : FF + (g + 1) * 1024,
                            ],
                        )
                        w1g.append(tg)
                    for fl in range(8):
                        fi = g * 8 + fl
                        a_ps = psum_f.tile([128, R], F32, tag="ffa")
                        g_ps = psum_f.tile([128, R], F32, tag="ffg")
                        for k in range(KT):
                            nc.tensor.matmul(
                                a_ps[:],
                                w1a[k][:, fl * 128 : (fl + 1) * 128],
                                xnT[k][:],
                                start=(k == 0),
                                stop=(k == KT - 1),
                            )
                        for k in range(KT):
                            nc.tensor.matmul(
                                g_ps[:],
                                w1g[k][:, fl * 128 : (fl + 1) * 128],
                                xnT[k][:],
                                start=(k == 0),
                                stop=(k == KT - 1),
                            )
                        sg = sg_pool.tile([128, R], F32, tag="sg")
                        nc.scalar.activation(
                            sg[:],
                            g_ps[:],
                            mybir.ActivationFunctionType.Sigmoid,
                        )
                        ag = sg_pool.tile([128, R], F32, tag="ag")
                        nc.vector.tensor_mul(ag[:], a_ps[:], sg[:])
                        nc.vector.tensor_mul(hT[fi][:], g_ps[:], ag[:])

            # ---- Phase D: fused Wo-proj + FF down-proj accumulation ----
            with (
                tc.tile_pool(name="wo", bufs=1) as wo_pool,
                tc.tile_pool(name="wff2", bufs=4) as wff2_pool,
                tc.tile_pool(name="ostage", bufs=2) as ostage,
                tc.tile_pool(name="psO", bufs=1, space="PSUM") as psum_o,
            ):
                wo_sb = [
                    wo_pool.tile([128, DIM], BF16, tag=f"wo{k}", name=f"wo{k}")
                    for k in range(KT)
                ]
                for k in range(KT):
                    nc.gpsimd.dma_start(
                        wo_sb[k][:], d_wo[k * 128 : (k + 1) * 128, :]
                    )
                o_ps = [
                    [psum_o.tile([128, 512], F32, tag=f"o{rs}{ch}", name=f"o{rs}{ch}") for ch in range(2)]
                    for rs in range(RT)
                ]
                for k in range(KT):
                    for rs in range(RT):
                        for ch in range(2):
                            nc.tensor.matmul(
                                o_ps[rs][ch][:],
                                aoT[k][:, rs * 128 : (rs + 1) * 128],
                                wo_sb[k][:, ch * 512 : (ch + 1) * 512],
                                start=(k == 0),
                                stop=False,
                            )
                for fi in range(FT):
                    w2 = wff2_pool.tile([128, DIM], BF16, tag="w2")
                    nc.gpsimd.dma_start(
                        w2[:], d_wff2[fi * 128 : (fi + 1) * 128, :]
                    )
                    for rs in range(RT):
                        for ch in range(2):
                            nc.tensor.matmul(
                                o_ps[rs][ch][:],
                                hT[fi][:, rs * 128 : (rs + 1) * 128],
                                w2[:, ch * 512 : (ch + 1) * 512],
                                start=False,
                                stop=(fi == FT - 1),
                            )
                for rs in range(RT):
                    o_sb = ostage.tile([128, DIM], F32, tag="ost")
                    for ch in range(2):
                        nc.scalar.copy(
                            o_sb[:, ch * 512 : (ch + 1) * 512], o_ps[rs][ch][:]
                        )
                    nc.gpsimd.dma_start(
                        d_out[rs * 128 : (rs + 1) * 128, :], o_sb[:]
                    )

    nc.compile()
    return nc


_NC_CACHE = {}


def _get_nc(x_bias_nonzero, c_bias_nonzero):
    key = (x_bias_nonzero, c_bias_nonzero)
    if key not in _NC_CACHE:
        _NC_CACHE[key] = build_kernel(*key)
    return _NC_CACHE[key]


def make_in_maps(x, context, norm_g, norm_b, cnorm_g, cnorm_b, Wq, Wkv, Wo, Wff1, Wff2):
    x = np.asarray(x, np.float32)
    context = np.asarray(context, np.float32)
    norm_g = np.asarray(norm_g, np.float32)
    norm_b = np.asarray(norm_b, np.float32)
    cnorm_g = np.asarray(cnorm_g, np.float32)
    cnorm_b = np.asarray(cnorm_b, np.float32)
    scale = DH ** -0.5
    bf = ml_dtypes.bfloat16
    wq = np.ascontiguousarray((norm_g[:, None] * np.asarray(Wq, np.float32)) * scale).astype(bf)
    wkv = np.ascontiguousarray(cnorm_g[:, None] * np.asarray(Wkv, np.float32)).astype(bf)
    wo = np.ascontiguousarray(np.asarray(Wo, np.float32)).astype(bf)
    wff1 = np.ascontiguousarray(norm_g[:, None] * np.asarray(Wff1, np.float32)).astype(bf)
    wff2 = np.ascontiguousarray(np.asarray(Wff2, np.float32)).astype(bf)
    x_bias = bool(np.any(norm_b != 0.0))
    c_bias = bool(np.any(cnorm_b != 0.0))
    in_maps = []
    for c in range(N_CORES):
        b = c // (N_CORES // B)
        r0 = (c % (N_CORES // B)) * R
        m = {
            "x": np.ascontiguousarray(x[b, r0 : r0 + R, :]),
            "ctx": np.ascontiguousarray(context[b]),
            "wq": wq,
            "wkv": wkv,
            "wo": wo,
            "wff1": wff1,
            "wff2": wff2,
        }
        if x_bias:
            m["xb"] = norm_b.reshape(1, DIM).copy()
        if c_bias:
            m["cb"] = cnorm_b.reshape(1, DIM).copy()
        in_maps.append(m)
    return in_maps, x_bias, c_bias


def gather_output(results):
    out = np.empty((B, N, DIM), np.float32)
    for c in range(N_CORES):
        b = c // (N_CORES // B)
        r0 = (c % (N_CORES // B)) * R
        out[b, r0 : r0 + R, :] = results[c]["out"]
    return out


def kernel(**inputs):
    from concourse.bass_utils import run_bass_kernel_spmd

    in_maps, x_bias, c_bias = make_in_maps(**inputs)
    nc = _get_nc(x_bias, c_bias)
    res = run_bass_kernel_spmd(nc, in_maps, list(range(N_CORES)))
    return gather_output(res.results)


# revision 9
# speedup vs baseline: 1.1039x; 1.1039x over previous
"""Trainium2 Bass kernel for nn_CrossAttention_65051574665735.

Cross-attention block (MQA, shared K/V head) + parallel SwiGLU FF.
Data-parallel over B*N rows across 8 NeuronCores: core c handles batch c//4,
rows (c%4)*512. Context + weights replicated (weights pre-cast to bf16 with the
layernorm scale g and the 1/sqrt(dh) attention scale folded in on the host).
No cross-core collectives; the host concatenates the 8 output slices.
"""

import sys

if "/opt/trn_rl_repo" not in sys.path:
    sys.path.insert(0, "/opt/trn_rl_repo")

import numpy as np
import ml_dtypes

import concourse.bass as bass
import concourse.tile as tile
from concourse import mybir, bacc
from concourse.masks import make_identity

F32 = mybir.dt.float32
BF16 = mybir.dt.bfloat16

B, N, J = 2, 2048, 2048
DIM, HEADS, DH = 1024, 16, 64
INNER = HEADS * DH
FF = 4 * DIM
EPS = 1e-5
N_CORES = 8
R = B * N // N_CORES  # 512 rows per core
KT = DIM // 128  # 8 contraction tiles over dim
RT = R // 128  # 4 row tiles
CT = J // 128  # 16 context row tiles
FT = FF // 128  # 32 ff tiles


def _layernorm_transpose(
    nc, tc, pools, src_dram, rows, xnT_tiles, bias_tile, tag
):
    """LN rows of src_dram ([rows, DIM] f32) and write transposed bf16 tiles.

    xnT_tiles: list of KT sbuf tiles [128, rows] bf16 (dim-major).
    bias_tile: optional [128, DIM] f32 sbuf tile holding broadcast norm_b.
    """
    ln_pool, stats_pool, psum_tr, ident, eps_tile = pools
    n_tiles = rows // 128
    for t in range(n_tiles):
        x_t = ln_pool.tile([128, DIM], F32, tag="ln_x")
        nc.gpsimd.dma_start(x_t[:], src_dram[t * 128 : (t + 1) * 128, :])
        stats = stats_pool.tile(
            [128, 2, nc.vector.BN_STATS_DIM], F32, tag="st"
        )
        nc.vector.bn_stats(stats[:, 0, :], x_t[:, 0:512])
        nc.vector.bn_stats(stats[:, 1, :], x_t[:, 512:1024])
        mv = stats_pool.tile([128, nc.vector.BN_AGGR_DIM], F32, tag="mv")
        nc.vector.bn_aggr(mv[:], stats[:])
        rstd = stats_pool.tile([128, 1], F32, tag="rs")
        nc.scalar.activation(
            rstd[:],
            mv[:, 1:2],
            mybir.ActivationFunctionType.Sqrt,
            bias=eps_tile[:],
        )
        nc.vector.reciprocal(rstd[:], rstd[:])
        xn_t = ln_pool.tile([128, DIM], BF16, tag="ln_xn")
        nc.vector.tensor_scalar(
            out=xn_t[:],
            in0=x_t[:],
            scalar1=mv[:, 0:1],
            scalar2=rstd[:],
            op0=mybir.AluOpType.subtract,
            op1=mybir.AluOpType.mult,
        )
        if bias_tile is not None:
            nc.vector.tensor_add(xn_t[:], xn_t[:], bias_tile[:])
        for k in range(KT):
            ps = psum_tr.tile([128, 128], BF16, tag="tr")
            nc.tensor.transpose(
                ps[:], xn_t[:, k * 128 : (k + 1) * 128], ident[:]
            )
            nc.vector.tensor_copy(
                xnT_tiles[k][:, t * 128 : (t + 1) * 128], ps[:]
            )


def build_kernel(x_bias_nonzero: bool, c_bias_nonzero: bool):
    nc = bacc.Bacc(
        "TRN2", target_bir_lowering=False, debug=False, num_devices=N_CORES
    )
    d_x = nc.dram_tensor("x", [R, DIM], F32, kind="ExternalInput").ap()
    d_ctx = nc.dram_tensor("ctx", [J, DIM], F32, kind="ExternalInput").ap()
    d_wq = nc.dram_tensor("wq", [DIM, INNER], BF16, kind="ExternalInput").ap()
    d_wkv = nc.dram_tensor("wkv", [DIM, 2 * DH], BF16, kind="ExternalInput").ap()
    d_wo = nc.dram_tensor("wo", [INNER, DIM], BF16, kind="ExternalInput").ap()
    d_wff1 = nc.dram_tensor("wff1", [DIM, 2 * FF], BF16, kind="ExternalInput").ap()
    d_wff2 = nc.dram_tensor("wff2", [FF, DIM], BF16, kind="ExternalInput").ap()
    d_xb = (
        nc.dram_tensor("xb", [1, DIM], F32, kind="ExternalInput").ap()
        if x_bias_nonzero
        else None
    )
    d_cb = (
        nc.dram_tensor("cb", [1, DIM], F32, kind="ExternalInput").ap()
        if c_bias_nonzero
        else None
    )
    d_out = nc.dram_tensor("out", [R, DIM], F32, kind="ExternalOutput").ap()

    with tile.TileContext(nc) as tc:
        with (
            tc.tile_pool(name="consts", bufs=1) as consts,
            tc.tile_pool(name="persist", bufs=1) as persist,
            tc.tile_pool(name="ln", bufs=3) as ln_pool,
            tc.tile_pool(name="stats", bufs=3) as stats_pool,
        ):
            ident = consts.tile([128, 128], BF16)
            make_identity(nc, ident)
            eps_tile = consts.tile([128, 1], F32, tag="eps")
            nc.vector.memset(eps_tile[:], EPS)

            xb_tile = cb_tile = None
            if d_xb is not None:
                xb_tile = consts.tile([128, DIM], F32, tag="xb")
                nc.gpsimd.dma_start(
                    xb_tile[:],
                    bass.AP(
                        tensor=d_xb.tensor, offset=d_xb.offset,
                        ap=[[0, 128]] + d_xb.ap[1:],
                    ),
                )
            if d_cb is not None:
                cb_tile = consts.tile([128, DIM], F32, tag="cb")
                nc.gpsimd.dma_start(
                    cb_tile[:],
                    bass.AP(
                        tensor=d_cb.tensor, offset=d_cb.offset,
                        ap=[[0, 128]] + d_cb.ap[1:],
                    ),
                )

            xnT = [
                persist.tile([128, R], BF16, tag=f"xnT{k}", name=f"xnT{k}")
                for k in range(KT)
            ]
            kT = persist.tile([128, J], BF16, tag="kT")
            vo = [
                persist.tile([128, DH + 1], BF16, tag=f"vo{j}", name=f"vo{j}")
                for j in range(CT)
            ]
            aoT = [
                persist.tile([128, R], BF16, tag=f"aoT{k}", name=f"aoT{k}")
                for k in range(KT)
            ]
            hT = [
                persist.tile([128, R], BF16, tag=f"hT{f}", name=f"hT{f}")
                for f in range(FT)
            ]

            # ---- Phase A+C: layernorms/transposes/KV, with the FF up-proj
            # emitted early so PE has dense work while the context LN runs on
            # DVE/ACT. PSUM budget: tr(2) + kv(2) + ffa/ffg(4) = 8 banks.
            with (
                tc.tile_pool(name="cnT", bufs=1) as cnT_pool,
                tc.tile_pool(name="psA", bufs=2, space="PSUM") as psum_tr,
                tc.tile_pool(name="psKV", bufs=2, space="PSUM") as psum_kv,
                tc.tile_pool(name="wkv", bufs=1) as wkv_pool,
                tc.tile_pool(name="vstage", bufs=2) as vstage,
                tc.tile_pool(name="wff1", bufs=24) as wff1_pool,
                tc.tile_pool(name="sg", bufs=3) as sg_pool,
                tc.tile_pool(name="psF", bufs=2, space="PSUM") as psum_f,
            ):
                pools = (ln_pool, stats_pool, psum_tr, ident, eps_tile)
                _layernorm_transpose(nc, tc, pools, d_x, R, xnT, xb_tile, "x")

                # FF up-proj + SwiGLU (needs only xnT + streamed wff1)
                for g in range(4):
                    w1a = []
                    w1g = []
                    for k in range(KT):
                        ta = wff1_pool.tile([128, 1024], BF16, tag="w1")
                        nc.sync.dma_start(
                            ta[:],
                            d_wff1[
                                k * 128 : (k + 1) * 128,
                                g * 1024 : (g + 1) * 1024,
                            ],
                        )
                        w1a.append(ta)
                        tg = wff1_pool.tile([128, 1024], BF16, tag="w1")
                        nc.sync.dma_start(
                            tg[:],
                            d_wff1[
                                k * 128 : (k + 1) * 128,
                                FF + g * 1024 : FF + (g + 1) * 1024,
                            ],
                        )
                        w1g.append(tg)
                    for fl in range(8):
                        fi = g * 8 + fl
                        a_ps = psum_f.tile([128, R], F32, tag="ffa")
                        g_ps = psum_f.tile([128, R], F32, tag="ffg")
                        for k in range(KT):
                            nc.tensor.matmul(
                                a_ps[:],
                                w1a[k][:, fl * 128 : (fl + 1) * 128],
                                xnT[k][:],
                                start=(k == 0),
                                stop=(k == KT - 1),
                            )
                        for k in range(KT):
                            nc.tensor.matmul(
                                g_ps[:],
                                w1g[k][:, fl * 128 : (fl + 1) * 128],
                                xnT[k][:],
                                start=(k == 0),
                                stop=(k == KT - 1),
                            )
                        sg = sg_pool.tile([128, R], F32, tag="sg")
                        nc.scalar.activation(
                            sg[:],
                            g_ps[:],
                            mybir.ActivationFunctionType.Sigmoid,
                        )
                        ag = sg_pool.tile([128, R], F32, tag="ag")
                        nc.vector.tensor_mul(ag[:], a_ps[:], sg[:])
                        nc.vector.tensor_mul(hT[fi][:], g_ps[:], ag[:])

                # context LN + transpose (mostly DVE/ACT; PE fills with FF)
                cnT = [
                    cnT_pool.tile([128, J], BF16, tag=f"cnT{k}", name=f"cnT{k}")
                    for k in range(KT)
                ]
                _layernorm_transpose(nc, tc, pools, d_ctx, J, cnT, cb_tile, "c")

                wkv_sb = [
                    wkv_pool.tile(
                        [128, 2 * DH], BF16, tag=f"wkv{k}", name=f"wkv{k}"
                    )
                    for k in range(KT)
                ]
                for k in range(KT):
                    nc.sync.dma_start(
                        wkv_sb[k][:], d_wkv[k * 128 : (k + 1) * 128, :]
                    )
                for jc in range(CT):
                    nc.vector.memset(vo[jc][:, DH : DH + 1], 1.0)

                for c in range(J // 512):
                    kv_ps = psum_kv.tile([128, 512], F32, tag="kv")
                    for k in range(KT):
                        nc.tensor.matmul(
                            kv_ps[:],
                            wkv_sb[k][:],
                            cnT[k][:, c * 512 : (c + 1) * 512],
                            start=(k == 0),
                            stop=(k == KT - 1),
                        )
                    # rows 0:64 = kT chunk; rows 64:128 = vT chunk
                    nc.scalar.copy(
                        kT[0:DH, c * 512 : (c + 1) * 512], kv_ps[0:DH, :]
                    )
                    nc.gpsimd.dma_start(
                        kT[DH:128, c * 512 : (c + 1) * 512],
                        kT[0:DH, c * 512 : (c + 1) * 512],
                    )
                    vT_sb = vstage.tile([128, 512], BF16, tag="vT")
                    nc.vector.tensor_copy(vT_sb[DH:128, :], kv_ps[DH:128, :])
                    for j4 in range(4):
                        jc = c * 4 + j4
                        vps = psum_tr.tile([128, DH], BF16, tag="tr")
                        nc.tensor.transpose(
                            vps[:],
                            vT_sb[DH:128, j4 * 128 : (j4 + 1) * 128],
                            ident[DH:128, DH:128],
                        )
                        nc.vector.tensor_copy(vo[jc][:, 0:DH], vps[:])

            # ---- Phase B: attention ----
            with (
                tc.tile_pool(name="wq", bufs=1) as wq_pool,
                tc.tile_pool(name="attnT", bufs=12) as attn_pool,
                tc.tile_pool(name="qT", bufs=2) as qT_pool,
                tc.tile_pool(name="smx", bufs=2) as smx_pool,
                tc.tile_pool(name="psQ", bufs=2, space="PSUM") as psum_q,
                tc.tile_pool(name="psS", bufs=2, space="PSUM") as psum_s,
                tc.tile_pool(name="psAV", bufs=2, space="PSUM") as psum_av,
            ):
                wq_sb = [
                    wq_pool.tile([128, INNER], BF16, tag=f"wq{k}", name=f"wq{k}")
                    for k in range(KT)
                ]
                for k in range(KT):
                    nc.sync.dma_start(
                        wq_sb[k][:], d_wq[k * 128 : (k + 1) * 128, :]
                    )
                for hp in range(HEADS // 2):
                    q_ps = psum_q.tile([128, R], F32, tag="q")
                    for k in range(KT):
                        nc.tensor.matmul(
                            q_ps[:],
                            wq_sb[k][:, hp * 128 : (hp + 1) * 128],
                            xnT[k][:],
                            start=(k == 0),
                            stop=(k == KT - 1),
                        )
                    qT_sb = qT_pool.tile([128, R], BF16, tag="qT")
                    nc.vector.tensor_copy(qT_sb[:], q_ps[:])
                    for h2 in range(2):
                        qh = qT_sb[h2 * 64 : (h2 + 1) * 64, :]
                        attnT = []
                        for jp in range(CT // 2):
                            s_ps = psum_s.tile([128, 2 * R], F32, tag="sim")
                            for half in range(2):
                                jc = 2 * jp + half
                                nc.tensor.matmul(
                                    s_ps[:, half * R : (half + 1) * R],
                                    kT[
                                        h2 * DH : (h2 + 1) * DH,
                                        jc * 128 : (jc + 1) * 128,
                                    ],
                                    qh,
                                    start=True,
                                    stop=True,
                                )
                            a_sb = attn_pool.tile([128, 2 * R], BF16, tag="attnT")
                            nc.scalar.activation(
                                a_sb[:],
                                s_ps[:],
                                mybir.ActivationFunctionType.Exp,
                            )
                            attnT.append(a_sb)
                        av_ps = psum_av.tile([DH + 1, R], F32, tag="av")
                        for jp in range(CT // 2):
                            for half in range(2):
                                jc = 2 * jp + half
                                nc.tensor.matmul(
                                    av_ps[:],
                                    vo[jc][:],
                                    attnT[jp][:, half * R : (half + 1) * R],
                                    start=(jc == 0),
                                    stop=(jc == CT - 1),
                                )
                        recip_t = smx_pool.tile([DH + 1, R], F32, tag="recipt")
                        nc.vector.reciprocal(
                            recip_t[DH : DH + 1, :], av_ps[DH : DH + 1, :]
                        )
                        recip = smx_pool.tile([1, R], F32, tag="recip")
                        nc.gpsimd.dma_start(recip[:], recip_t[DH : DH + 1, :])
                        rbc = smx_pool.tile([DH, R], F32, tag="rbc")
                        nc.gpsimd.partition_broadcast(rbc[:], recip[:])
                        if h2 == 0:
                            nc.vector.tensor_mul(
                                aoT[hp][0:DH, :], av_ps[0:DH, :], rbc[:]
                            )
                        else:
                            tmp = smx_pool.tile([DH, R], BF16, tag="aotmp")
                            nc.vector.tensor_mul(
                                tmp[:], av_ps[0:DH, :], rbc[:]
                            )
                            nc.gpsimd.dma_start(aoT[hp][DH:128, :], tmp[:])

            # ---- Phase D: fused Wo-proj + FF down-proj accumulation ----
            with (
                tc.tile_pool(name="wo", bufs=1) as wo_pool,
                tc.tile_pool(name="wff2", bufs=6) as wff2_pool,
                tc.tile_pool(name="ostage", bufs=2) as ostage,
                tc.tile_pool(name="psO", bufs=1, space="PSUM") as psum_o,
            ):
                wo_sb = [
                    wo_pool.tile([128, DIM], BF16, tag=f"wo{k}", name=f"wo{k}")
                    for k in range(KT)
                ]
                for k in range(KT):
                    nc.sync.dma_start(
                        wo_sb[k][:], d_wo[k * 128 : (k + 1) * 128, :]
                    )
                o_ps = [
                    [
                        psum_o.tile(
                            [128, 512], F32, tag=f"o{rs}{ch}", name=f"o{rs}{ch}"
                        )
                        for ch in range(2)
                    ]
                    for rs in range(RT)
                ]
                for k in range(KT):
                    for rs in range(RT):
                        for ch in range(2):
                            nc.tensor.matmul(
                                o_ps[rs][ch][:],
                                aoT[k][:, rs * 128 : (rs + 1) * 128],
                                wo_sb[k][:, ch * 512 : (ch + 1) * 512],
                                start=(k == 0),
                                stop=False,
                            )
                for fi in range(FT):
                    w2 = wff2_pool.tile([128, DIM], BF16, tag="w2")
                    nc.sync.dma_start(
                        w2[:], d_wff2[fi * 128 : (fi + 1) * 128, :]
                    )
                    for rs in range(RT):
                        for ch in range(2):
                            nc.tensor.matmul(
                                o_ps[rs][ch][:],
                                hT[fi][:, rs * 128 : (rs + 1) * 128],
                                w2[:, ch * 512 : (ch + 1) * 512],
                                start=False,
                                stop=(fi == FT - 1),
                            )
                for rs in range(RT):
                    o_sb = ostage.tile([128, DIM], F32, tag="ost")
                    for ch in range(2):
                        nc.scalar.copy(
                            o_sb[:, ch * 512 : (ch + 1) * 512], o_ps[rs][ch][:]
                        )
                    nc.gpsimd.dma_start(
                        d_out[rs * 128 : (rs + 1) * 128, :], o_sb[:]
                    )

    nc.compile()
    return nc


_NC_CACHE = {}


def _get_nc(x_bias_nonzero, c_bias_nonzero):
    key = (x_bias_nonzero, c_bias_nonzero)
    if key not in _NC_CACHE:
        _NC_CACHE[key] = build_kernel(*key)
    return _NC_CACHE[key]


def make_in_maps(x, context, norm_g, norm_b, cnorm_g, cnorm_b, Wq, Wkv, Wo, Wff1, Wff2):
    x = np.asarray(x, np.float32)
    context = np.asarray(context, np.float32)
    norm_g = np.asarray(norm_g, np.float32)
    norm_b = np.asarray(norm_b, np.float32)
    cnorm_g = np.asarray(cnorm_g, np.float32)
    cnorm_b = np.asarray(cnorm_b, np.float32)
    scale = DH ** -0.5
    bf = ml_dtypes.bfloat16
    wq = np.ascontiguousarray((norm_g[:, None] * np.asarray(Wq, np.float32)) * scale).astype(bf)
    wkv = np.ascontiguousarray(cnorm_g[:, None] * np.asarray(Wkv, np.float32)).astype(bf)
    wo = np.ascontiguousarray(np.asarray(Wo, np.float32)).astype(bf)
    wff1 = np.ascontiguousarray(norm_g[:, None] * np.asarray(Wff1, np.float32)).astype(bf)
    wff2 = np.ascontiguousarray(np.asarray(Wff2, np.float32)).astype(bf)
    x_bias = bool(np.any(norm_b != 0.0))
    c_bias = bool(np.any(cnorm_b != 0.0))
    in_maps = []
    for c in range(N_CORES):
        b = c // (N_CORES // B)
        r0 = (c % (N_CORES // B)) * R
        m = {
            "x": np.ascontiguousarray(x[b, r0 : r0 + R, :]),
            "ctx": np.ascontiguousarray(context[b]),
            "wq": wq,
            "wkv": wkv,
            "wo": wo,
            "wff1": wff1,
            "wff2": wff2,
        }
        if x_bias:
            m["xb"] = norm_b.reshape(1, DIM).copy()
        if c_bias:
            m["cb"] = cnorm_b.reshape(1, DIM).copy()
        in_maps.append(m)
    return in_maps, x_bias, c_bias


def gather_output(results):
    out = np.empty((B, N, DIM), np.float32)
    for c in range(N_CORES):
        b = c // (N_CORES // B)
        r0 = (c % (N_CORES // B)) * R
        out[b, r0 : r0 + R, :] = results[c]["out"]
    return out


def kernel(**inputs):
    from concourse.bass_utils import run_bass_kernel_spmd

    in_maps, x_bias, c_bias = make_in_maps(**inputs)
    nc = _get_nc(x_bias, c_bias)
    res = run_bass_kernel_spmd(nc, in_maps, list(range(N_CORES)))
    return gather_output(res.results)
